# revision 14
# baseline (speedup 1.0000x reference)
"""Trainium2 Bass kernel for nn_AttentionBlock (self-attn + cross-attn block).

Sharding: 8 cores = (2 batches) x (4 query-chunks of 512). Zero communication:
each core recomputes the kv projection for its batch (on-chip collectives are
far slower than the duplicated matmul work). Inside a core everything runs in
"transposed" orientation:

  xnT = LN(x)^T                [feat, ctx]   (PE transpose of centered rows,
                                              per-feature affine applied after)
  qT  = (wq*S)^T @ xnT_q       [qcol, nq]    (softmax scale folded into wq)
  kT  = wk^T @ xnT_c           [kcol, nk]    (streamed per 128-col chunk)
  v   = xnT_c^T @ wv           [nk, vcol]    (natural orientation, 66-wide head
                                              slots with a ones column at 64)
  simT[k, q] = kT_h (stationary) x qT_h (moving)   -> PSUM f32
  attn = exp(simT)  (ACT, no max-subtraction: |S*sim| < ~4)
  attn *= exp(S*bias)   (DVE, only on the non-saturated diagonal band)
  avT[0:66, q] = sum_k v_slot[k, :] * attn[k, q]   (row 64 = softmax denom)
  oT[h*64+d, q] = avT[d, q] * recip(avT[64, q])
  out[q, f] = oT^T @ wo + bo + residual

T5 rel-pos bias enters multiplicatively: exp((sim+bias)*S) =
exp(S*sim)*exp(S*bias), and bias depends only on (k - q), saturating for
|k - q| >= 91. The host rotates the self-attn context rows per core so the
non-saturated band sits at fixed k-tiles 5..10; those tiles get a DVE multiply
with host-computed exp(S*bias) tiles, every other tile folds its constant
bias factor into the v rows (applied during the v-projection PSUM->SBUF copy).
The cross-attn band position varies per core, so all 4 cross k-tiles use the
DVE multiply path.
"""

import sys

sys.path.insert(0, "/opt/trn_rl_repo")

import numpy as np
import ml_dtypes

BF16 = ml_dtypes.bfloat16

P = 128
F = 1024
FC = 8  # feature chunks of 128
H = 16
D = 64
NCTX = 2048
NQ = 512
MCTX = 512
NB = 32
MAXD = 128
EPS = 1e-5
S = np.float32(D) ** np.float32(-0.5)
ROT = 768  # own q rows sit at rotated positions [768, 1280)
BAND0 = 5  # self-attn band tiles 5..10
NBAND = 6
SLOT = 66  # v head slot: 64 v cols + ones col + pad
NKT = NCTX // P  # 16 self k-tiles
NKTC = MCTX // P  # 4 cross k-tiles

_CACHE = {}


def _bucket_np(rel):
    """T5 relative position bucket, non-causal. Must match reference.py."""
    rel = np.asarray(rel, np.int64)
    nb = NB // 2
    ret = (rel >= 0).astype(np.int32) * nb
    n = np.abs(rel)
    max_exact = nb // 2
    val_large = max_exact + (
        np.log(np.maximum(n, 1).astype(np.float32) / np.float32(max_exact))
        / np.float32(np.log(MAXD / max_exact))
        * np.float32(nb - max_exact)
    ).astype(np.int32)
    val_large = np.minimum(val_large, nb - 1)
    return ret + np.where(n < max_exact, n, val_large)


def _build_module():
    import concourse.bass as bass
    import concourse.tile as tile
    import concourse.mybir as mybir
    from concourse import bacc
    from contextlib import ExitStack

    f32 = mybir.dt.float32
    bf16 = mybir.dt.bfloat16
    AF = mybir.ActivationFunctionType
    ALU = mybir.AluOpType

    nc = bacc.Bacc("TRN2", target_bir_lowering=False, debug=False, num_devices=8)

    # ---- DRAM I/O ----
    d_xctx = nc.dram_tensor("x_ctx", [NCTX, F], f32, kind="ExternalInput").ap()
    d_ctx = nc.dram_tensor("ctx", [MCTX, F], f32, kind="ExternalInput").ap()
    dw = {}
    for nm in ("w_qT", "w_kT", "w_v", "w_o", "cw_qT", "cw_kT", "cw_v", "cw_o"):
        dw[nm] = nc.dram_tensor(nm, [FC, P, F], bf16, kind="ExternalInput").ap()
    # norm vectors, feat-chunked per partition: [vec, 128, FC]
    # order: 0=sa_gc 1=sa_bc 2=sa_g 3=sa_b 4=ca_g 5=ca_b 6=ca_gc 7=ca_bc
    d_nrm = nc.dram_tensor("nrm", [8, P, FC], f32, kind="ExternalInput").ap()
    d_bo = nc.dram_tensor("bo2", [2, P, F], f32, kind="ExternalInput").ap()
    d_ebs = nc.dram_tensor(
        "eb_self", [H, NBAND, P, NQ], bf16, kind="ExternalInput"
    ).ap()
    d_ebc = nc.dram_tensor(
        "eb_cross", [H, NKTC, P, NQ], bf16, kind="ExternalInput"
    ).ap()
    d_vfac = nc.dram_tensor("vfac", [NKT, 2, P, 512], bf16, kind="ExternalInput").ap()
    d_ones = nc.dram_tensor("ones_map", [P, NKT, H], bf16, kind="ExternalInput").ap()
    d_onesc = nc.dram_tensor("ones_c", [P, NKTC, H], bf16, kind="ExternalInput").ap()
    d_ident = nc.dram_tensor("ident", [P, P], bf16, kind="ExternalInput").ap()
    d_y = nc.dram_tensor("y", [NQ, F], f32, kind="ExternalOutput").ap()
    d_x2 = nc.dram_tensor("x2d", [NQ, F], f32).ap()  # internal scratch

    es = ExitStack()
    tc = es.enter_context(tile.TileContext(nc))

    pool = lambda name, bufs, **kw: es.enter_context(
        tc.tile_pool(name=name, bufs=bufs, **kw)
    )
    px = pool("px", 2)  # x input tiles f32 [128,1024]
    pst = pool("pst", 6)  # small stats tiles
    pcent = pool("pcent", 5)  # centered bf16 [128,1024]
    pwres = pool("pwres", 8)  # resident weight slabs (qT/v/out phases)
    pwk = pool("pwk", 8)  # resident kT weight slabs
    pkT = pool("pkT", 2)  # kT chunks [128, 2048] bf16
    pattn = pool("pattn", 4)  # attn group tiles [128, 1024] bf16
    peb = pool("peb", 4)  # eb tiles bf16 [128,512]
    pvf = pool("pvf", 2)  # vfac tiles bf16 [128,512]
    pres = pool("pres", 2)  # residual tiles f32 [128,512]
    pyt = pool("pyt", 2)  # output tiles f32 [128,512]
    pden = pool("pden", 4)  # denom tiles
    pone = pool("pone", 2)  # ones map tiles
    pmm = pool("pmm", 2, space="PSUM")  # [128,512] proj/transpose psum: 2 banks
    psim = pool("psim", 2, space="PSUM")  # [128,1024] sim psum: 4 banks
    pav = pool("pav", 2, space="PSUM")  # [66,512] av psum: 2 banks
    pbig = pool("pbig", 1)  # persistent tensors (one slot per tag)

    def big(name, shape, dtype, tag=None):
        return pbig.tile(shape, dtype, tag=tag or name, name=name)

    # persistent SBUF (self phase; cross phase reuses the same tags)
    xnTc = big("xnTc", [P, FC, NCTX], bf16)
    xnTq = big("xnTq", [P, FC, NQ], bf16)
    qT = big("qT", [P, FC, NQ], bf16)
    vsb = big("vsb", [P, NKT, H * SLOT], bf16)
    oT = big("oT", [P, FC, NQ], bf16)
    bo_s = big("bo_s", [P, F], f32)
    bo_c = big("bo_c", [P, F], f32)
    ident = big("ident", [P, P], bf16)
    nrmv = big("nrmv", [P, 8, FC], f32)

    eps_t = big("eps", [P, 1], f32)
    nc.vector.memset(eps_t[:, :], float(EPS))
    ones64 = big("ones64", [1, D], f32)
    nc.vector.memset(ones64[:, :], 1.0)
    nc.sync.dma_start(out=ident[:, :], in_=d_ident[:, :])
    nc.sync.dma_start(out=bo_s[:, :], in_=d_bo[0])
    nc.sync.dma_start(out=bo_c[:, :], in_=d_bo[1])
    nc.sync.dma_start(out=nrmv[:, :, :], in_=d_nrm.rearrange("v p c -> p v c"))

    def g_ap(vi, fc):
        return nrmv[:, vi, fc : fc + 1]

    # ---------------- layer norm + transpose ----------------
    def ln_transpose(x_dram, r0, nrows, affines):
        """LN rows [r0, r0+nrows) of x_dram; write transposed+affined copies.

        affines: list of (dst, vi_g, vi_b, src_lo, src_hi) — dst[:, fc, j]
        gets position src_lo+j, for src positions within [src_lo, src_hi).
        Positions are local (0 = row r0).
        """
        nt = nrows // P
        for g0 in range(0, nt, 4):
            gn = min(4, nt - g0)
            cents = []
            for it in range(g0, g0 + gn):
                xt = px.tile([P, F], f32, tag="xt")
                nc.sync.dma_start(
                    out=xt[:, :], in_=x_dram[r0 + it * P : r0 + (it + 1) * P, :]
                )
                st6 = pst.tile([P, 2, 6], f32, tag="st6")
                nc.vector.bn_stats(st6[:, 0, :], xt[:, 0:512])
                nc.vector.bn_stats(st6[:, 1, :], xt[:, 512:1024])
                mv = pst.tile([P, 2], f32, tag="mv")
                nc.vector.bn_aggr(mv[:, :], st6[:, :, :])
                std = pst.tile([P, 1], f32, tag="std")
                nc.scalar.activation(std[:, :], mv[:, 1:2], AF.Sqrt, bias=eps_t[:, 0:1])
                rstd = pst.tile([P, 1], f32, tag="rstd")
                nc.vector.reciprocal(rstd[:, :], std[:, :])
                nmr = pst.tile([P, 1], f32, tag="nmr")
                nc.vector.scalar_tensor_tensor(
                    nmr[:, :], mv[:, 0:1], -1.0, rstd[:, :], ALU.mult, ALU.mult
                )
                cent = pcent.tile([P, F], bf16, tag="cent")
                nc.scalar.activation(
                    cent[:, :],
                    xt[:, :],
                    AF.Identity,
                    bias=nmr[:, 0:1],
                    scale=rstd[:, 0:1],
                )
                cents.append(cent)
            base = g0 * P
            for fc in range(FC):
                ps = pmm.tile([P, 4 * P], bf16, tag="mm")
                for j in range(gn):
                    nc.tensor.transpose(
                        ps[:, j * P : (j + 1) * P],
                        cents[j][:, fc * P : (fc + 1) * P],
                        ident[:, :],
                    )
                for dst, vig, vib, lo, hi in affines:
                    lo2, hi2 = max(lo, base), min(hi, base + gn * P)
                    if lo2 >= hi2:
                        continue
                    nc.vector.tensor_scalar(
                        dst[:, fc, lo2 - lo : hi2 - lo],
                        ps[:, lo2 - base : hi2 - base],
                        g_ap(vig, fc),
                        g_ap(vib, fc),
                        ALU.mult,
                        ALU.add,
                    )

    # ---------------- projection helpers ----------------
    def load_slabs(w_dram, pw, tag):
        slabs = []
        for k in range(FC):
            ws = pw.tile([P, F], bf16, tag=tag)
            nc.sync.dma_start(out=ws[:, :], in_=w_dram[k])
            slabs.append(ws)
        return slabs

    def proj_T(dst, w_dram, src, n_cols):
        """dst[:, m, :] = w^T @ src  (both operands feat-major chunked)."""
        slabs = load_slabs(w_dram, pwres, "wres")
        for m in range(FC):
            for n0 in range(0, n_cols, 512):
                nn = min(512, n_cols - n0)
                ps = pmm.tile([P, 512], f32, tag="mm")
                for k in range(FC):
                    nc.tensor.matmul(
                        ps[:, 0:nn],
                        slabs[k][:, m * P : (m + 1) * P],
                        src[:, k, n0 : n0 + nn],
                        start=(k == 0),
                        stop=(k == FC - 1),
                    )
                nc.vector.tensor_copy(dst[:, m, n0 : n0 + nn], ps[:, 0:nn])

    def v_proj(dst, w_dram, src, nkt, use_fac, ones_dram):
        """dst [P, nkt, H*SLOT]: natural-orientation v with slotted heads."""
        slabs = load_slabs(w_dram, pwres, "wres")
        for n in range(2):  # vcol halves: heads 0-7 / 8-15
            for m in range(nkt):
                ps = pmm.tile([P, 512], f32, tag="mm")
                for k in range(FC):
                    nc.tensor.matmul(
                        ps[:, :],
                        src[:, k, m * P : (m + 1) * P],
                        slabs[k][:, n * 512 : (n + 1) * 512],
                        start=(k == 0),
                        stop=(k == FC - 1),
                    )
                out_ap = dst[:, m, n * 8 * SLOT : (n + 1) * 8 * SLOT].rearrange(
                    "p (h s) -> p h s", h=8
                )[:, :, 0:D]
                in_ap = ps[:, :].rearrange("p (h s) -> p h s", h=8)
                if use_fac:
                    vf = pvf.tile([P, 512], bf16, tag="vf")
                    nc.sync.dma_start(out=vf[:, :], in_=d_vfac[m, n])
                    nc.vector.tensor_tensor(
                        out_ap,
                        in_ap,
                        vf[:, :].rearrange("p (h s) -> p h s", h=8),
                        ALU.mult,
                    )
                else:
                    nc.vector.tensor_copy(out_ap, in_ap)
        ones_t = pone.tile([P, nkt, H], bf16, tag="ones")
        nc.sync.dma_start(out=ones_t[:, :, :], in_=ones_dram[:, 0:nkt, :])
        nc.vector.tensor_copy(
            dst[:, 0:nkt, :].rearrange("p t (h s) -> p t h s", h=H)[:, :, :, D],
            ones_t[:, :, :],
        )

    def kT_mchunk(slabs, src, m, nctx):
        """kT chunk [128, nctx] for kcol tile m (heads 2m, 2m+1)."""
        kt = pkT.tile([P, NCTX], bf16, tag="kT")
        for n0 in range(0, nctx, 512):
            ps = pmm.tile([P, 512], f32, tag="mm")
            for k in range(FC):
                nc.tensor.matmul(
                    ps[:, :],
                    slabs[k][:, m * P : (m + 1) * P],
                    src[:, k, n0 : n0 + 512],
                    start=(k == 0),
                    stop=(k == FC - 1),
                )
            nc.vector.tensor_copy(kt[:, n0 : n0 + 512], ps[:, :])
        return kt

    def attention(h, kt, qT_t, vsb_t, oT_t, nkt, eb_dram, band_lo, band_hi):
        """One head: sim -> exp -> (band mul) -> av -> normalized oT slice."""
        po = 64 * (h % 2)
        mc = h // 2
        gtiles = []
        for g0 in range(0, nkt, 2):
            ps = psim.tile([P, 1024], f32, tag="sim")
            for j in range(2):
                kc = g0 + j
                nc.tensor.matmul(
                    ps[:, j * 512 : (j + 1) * 512],
                    kt[po : po + D, kc * P : (kc + 1) * P],
                    qT_t[po : po + D, mc, :],
                    start=True,
                    stop=True,
                )
            at = pattn.tile([P, 1024], bf16, tag="attn")
            nc.scalar.activation(at[:, :], ps[:, :], AF.Exp)
            for j in range(2):
                kc = g0 + j
                if band_lo <= kc < band_hi:
                    ebt = peb.tile([P, 512], bf16, tag="ebt")
                    nc.sync.dma_start(out=ebt[:, :], in_=eb_dram[h, kc - band_lo])
                    nc.vector.tensor_tensor(
                        at[:, j * 512 : (j + 1) * 512],
                        at[:, j * 512 : (j + 1) * 512],
                        ebt[:, :],
                        ALU.mult,
                    )
            gtiles.append(at)
        pso = pav.tile([D + 1, 512], f32, tag="av")
        for kc in range(nkt):
            nc.tensor.matmul(
                pso[:, :],
                vsb_t[:, kc, h * SLOT : h * SLOT + D + 1],
                gtiles[kc // 2][:, (kc % 2) * 512 : (kc % 2 + 1) * 512],
                start=(kc == 0),
                stop=(kc == nkt - 1),
            )
        f32r = mybir.dt.float32r
        den = pden.tile([1, 512], f32r, tag="den")
        with nc.allow_low_precision(reason="softmax denom reciprocal in f32r"):
            nc.vector.reciprocal(den[:, :], pso[D : D + 1, :])
        psb = pmm.tile([P, 512], f32, tag="mm")
        nc.tensor.matmul(
            psb[0:D, :],
            ones64[:, :].bitcast(f32r),
            den[:, :],
            start=True,
            stop=True,
        )
        denb = pden.tile([D, 512], f32, tag="denb")
        nc.scalar.copy(denb[:, :], psb[0:D, :])
        nc.vector.tensor_tensor(
            oT_t[po : po + D, mc, :], pso[0:D, :], denb[:, :], ALU.mult
        )

    def out_proj(oT_t, w_dram, bo_t, dst_fn):
        slabs = load_slabs(w_dram, pwres, "wres")
        for qm in range(4):
            for n in range(2):
                ps = pmm.tile([P, 512], f32, tag="mm")
                for k in range(FC):
                    nc.tensor.matmul(
                        ps[:, :],
                        oT_t[:, k, qm * P : (qm + 1) * P],
                        slabs[k][:, n * 512 : (n + 1) * 512],
                        start=(k == 0),
                        stop=(k == FC - 1),
                    )
                tmp = pyt.tile([P, 512], f32, tag="yt")
                nc.vector.tensor_tensor(
                    tmp[:, :], ps[:, :], bo_t[:, n * 512 : (n + 1) * 512], ALU.add
                )
                dst_fn(qm, n, tmp)

    # ================= self attention =================
    ln_transpose(
        d_xctx, 0, NCTX, [(xnTc, 0, 1, 0, NCTX), (xnTq, 2, 3, ROT, ROT + NQ)]
    )
    proj_T(qT, dw["w_qT"], xnTq, NQ)
    v_proj(vsb, dw["w_v"], xnTc, NKT, True, d_ones)

    wk_slabs = load_slabs(dw["w_kT"], pwk, "wk")
    for m in range(FC):
        kt = kT_mchunk(wk_slabs, xnTc, m, NCTX)
        for h in (2 * m, 2 * m + 1):
            attention(h, kt, qT, vsb, oT, NKT, d_ebs, BAND0, BAND0 + NBAND)

    def self_dst(qm, n, tmp):
        rt = pres.tile([P, 512], f32, tag="res")
        nc.sync.dma_start(
            out=rt[:, :],
            in_=d_xctx[
                ROT + qm * P : ROT + (qm + 1) * P, n * 512 : (n + 1) * 512
            ],
        )
        x2t = pyt.tile([P, 512], f32, tag="yt")
        nc.vector.tensor_tensor(x2t[:, :], tmp[:, :], rt[:, :], ALU.add)
        nc.sync.dma_start(
            out=d_x2[qm * P : (qm + 1) * P, n * 512 : (n + 1) * 512], in_=x2t[:, :]
        )

    out_proj(oT, dw["w_o"], bo_s, self_dst)

    # ================= cross attention =================
    ctxnT = big("ctxnT", [P, FC, MCTX], bf16, tag="xnTc")
    x2nT = big("x2nT", [P, FC, NQ], bf16, tag="xnTq")
    qTc = big("qTc", [P, FC, NQ], bf16, tag="qT")
    vc = big("vc", [P, NKTC, H * SLOT], bf16, tag="vsb")
    oTc = big("oTc", [P, FC, NQ], bf16, tag="oT")

    ln_transpose(d_ctx, 0, MCTX, [(ctxnT, 6, 7, 0, MCTX)])
    ln_transpose(d_x2, 0, NQ, [(x2nT, 4, 5, 0, NQ)])
    proj_T(qTc, dw["cw_qT"], x2nT, NQ)
    v_proj(vc, dw["cw_v"], ctxnT, NKTC, False, d_onesc)

    wkc_slabs = load_slabs(dw["cw_kT"], pwk, "wk")
    for m in range(FC):
        kt = kT_mchunk(wkc_slabs, ctxnT, m, MCTX)
        for h in (2 * m, 2 * m + 1):
            attention(h, kt, qTc, vc, oTc, NKTC, d_ebc, 0, NKTC)

    def cross_dst(qm, n, tmp):
        rt = pres.tile([P, 512], f32, tag="res")
        nc.sync.dma_start(
            out=rt[:, :], in_=d_x2[qm * P : (qm + 1) * P, n * 512 : (n + 1) * 512]
        )
        yt = pyt.tile([P, 512], f32, tag="yt")
        nc.vector.tensor_tensor(yt[:, :], tmp[:, :], rt[:, :], ALU.add)
        nc.sync.dma_start(
            out=d_y[qm * P : (qm + 1) * P, n * 512 : (n + 1) * 512], in_=yt[:, :]
        )

    out_proj(oTc, dw["cw_o"], bo_c, cross_dst)

    es.close()
    nc.compile()
    return nc


# ---------------------------------------------------------------------------
# host side
# ---------------------------------------------------------------------------


def _prep_shared(inputs):
    sh = {}
    for pre in ("sa", "ca"):
        wq = inputs[f"{pre}_wq"].astype(np.float32) * S
        wkv = inputs[f"{pre}_wkv"].astype(np.float32)
        wo = inputs[f"{pre}_wo"].astype(np.float32)
        sh[f"{pre}_wqT"] = np.ascontiguousarray(wq.reshape(FC, P, F).astype(BF16))
        sh[f"{pre}_wkT"] = np.ascontiguousarray(
            wkv[:, :F].reshape(FC, P, F).astype(BF16)
        )
        sh[f"{pre}_wv"] = np.ascontiguousarray(wkv[:, F:].reshape(FC, P, F).astype(BF16))
        sh[f"{pre}_wo"] = np.ascontiguousarray(wo.reshape(FC, P, F).astype(BF16))
    nrm = np.stack(
        [
            inputs["sa_normc_g"],
            inputs["sa_normc_b"],
            inputs["sa_norm_g"],
            inputs["sa_norm_b"],
            inputs["ca_norm_g"],
            inputs["ca_norm_b"],
            inputs["ca_normc_g"],
            inputs["ca_normc_b"],
        ]
    ).astype(np.float32)
    # [8, F] -> [8, P, FC]: value for (partition p, chunk c) = vec[c*128+p]
    sh["nrm"] = np.ascontiguousarray(nrm.reshape(8, FC, P).transpose(0, 2, 1))
    sh["bo2"] = np.ascontiguousarray(
        np.stack(
            [
                np.broadcast_to(inputs["sa_bo"].astype(np.float32), (P, F)),
                np.broadcast_to(inputs["ca_bo"].astype(np.float32), (P, F)),
            ]
        )
    )
    sh["ident"] = np.eye(P, dtype=np.float32).astype(BF16)
    sh["ones_c"] = np.ones((P, NKTC, H), BF16)
    return sh


def _prep_core(inputs, sh, b, qi):
    q0 = qi * NQ
    x = np.asarray(inputs["x"][b], np.float32)
    ctx = np.asarray(inputs["context"][b], np.float32)
    rot = (np.arange(NCTX) + (q0 - ROT)) % NCTX  # position i <- orig row rot[i]
    x_rot = np.ascontiguousarray(x[rot])

    emb_s = np.asarray(inputs["sa_rel"], np.float32)  # [32, 16]
    qcols = q0 + np.arange(NQ)
    pos = np.arange(BAND0 * P, (BAND0 + NBAND) * P)
    rel = rot[pos][:, None] - qcols[None, :]
    eb = np.exp(S * emb_s[_bucket_np(rel)])  # [NBAND*P, NQ, H]
    eb_self = np.ascontiguousarray(
        eb.transpose(2, 0, 1).reshape(H, NBAND, P, NQ).astype(BF16)
    )
    vfac = np.ones((NKT, H), np.float32)
    for t in list(range(0, BAND0)) + list(range(BAND0 + NBAND, NKT)):
        pos_t = np.arange(t * P, (t + 1) * P)
        rel_t = rot[pos_t][:, None] - qcols[None, :]
        bk = _bucket_np(rel_t)
        assert bk.min() == bk.max(), (b, qi, t, bk.min(), bk.max())
        vfac[t] = np.exp(S * emb_s[bk[0, 0]])
    # vfac dram layout [NKT, 2, P, 512]: [m, n, p, vcol] with vcol = 8 heads x 64
    vfac_t = np.repeat(vfac, D, axis=1).reshape(NKT, 2, 512)  # [m, n, 512]
    vfac_full = np.broadcast_to(vfac_t[:, :, None, :], (NKT, 2, P, 512)).astype(BF16)
    ones_map = np.broadcast_to(vfac[None, :, :], (P, NKT, H)).astype(BF16)

    emb_c = np.asarray(inputs["ca_rel"], np.float32)
    relc = np.arange(MCTX)[:, None] - qcols[None, :] + (NCTX - MCTX)
    ebc = np.exp(S * emb_c[_bucket_np(relc)])
    eb_cross = np.ascontiguousarray(
        ebc.transpose(2, 0, 1).reshape(H, NKTC, P, NQ).astype(BF16)
    )

    return {
        "x_ctx": x_rot,
        "ctx": np.ascontiguousarray(ctx),
        "w_qT": sh["sa_wqT"],
        "w_kT": sh["sa_wkT"],
        "w_v": sh["sa_wv"],
        "w_o": sh["sa_wo"],
        "cw_qT": sh["ca_wqT"],
        "cw_kT": sh["ca_wkT"],
        "cw_v": sh["ca_wv"],
        "cw_o": sh["ca_wo"],
        "nrm": sh["nrm"],
        "bo2": sh["bo2"],
        "eb_self": eb_self,
        "eb_cross": eb_cross,
        "vfac": np.ascontiguousarray(vfac_full),
        "ones_map": np.ascontiguousarray(ones_map),
        "ones_c": sh["ones_c"],
        "ident": sh["ident"],
    }


def get_module():
    if "nc" not in _CACHE:
        _CACHE["nc"] = _build_module()
    return _CACHE["nc"]


def prep_all_cores(inputs):
    sh = _prep_shared(inputs)
    return [_prep_core(inputs, sh, *divmod(core, 4)) for core in range(8)]


def kernel(**inputs):
    from concourse.bass_utils import run_bass_kernel_spmd

    nc = get_module()
    in_maps = prep_all_cores(inputs)
    res = run_bass_kernel_spmd(nc, in_maps, list(range(8)))
    out = np.empty((2, NCTX, F), np.float32)
    for core in range(8):
        b, qi = divmod(core, 4)
        out[b, qi * NQ : (qi + 1) * NQ] = res.results[core]["y"]
    return out


# revision 26
# speedup vs baseline: 175.3227x; 175.3227x over previous
"""Trainium2 Bass kernel for nn_AttentionBlock (self-attn + cross-attn block).

Sharding: 8 cores = (2 batches) x (4 query-chunks of 512). Zero communication:
each core recomputes the kv projection for its batch (on-chip collectives are
far slower than the duplicated matmul work). Inside a core everything runs in
"transposed" orientation:

  xnT = LN(x)^T                [feat, ctx]   (PE transpose of centered rows,
                                              per-feature affine applied after)
  qT  = (wq*S)^T @ xnT_q       [qcol, nq]    (softmax scale folded into wq)
  kT  = wk^T @ xnT_c           [kcol, nk]    (streamed per 128-col chunk)
  v   = xnT_c^T @ wv           [nk, vcol]    (natural orientation, 66-wide head
                                              slots with a ones column at 64)
  simT[k, q] = kT_h (stationary) x qT_h (moving)   -> PSUM f32
  attn = exp(simT)  (ACT, no max-subtraction: |S*sim| < ~4)
  attn *= exp(S*bias)   (DVE, only on the non-saturated diagonal band)
  avT[0:66, q] = sum_k v_slot[k, :] * attn[k, q]   (row 64 = softmax denom)
  oT[h*64+d, q] = avT[d, q] * recip(avT[64, q])
  out[q, f] = oT^T @ wo + bo + residual

T5 rel-pos bias enters multiplicatively: exp((sim+bias)*S) =
exp(S*sim)*exp(S*bias), and bias depends only on (k - q), saturating for
|k - q| >= 91. The host rotates the self-attn context rows per core so the
non-saturated band sits at fixed k-tiles 5..10; those tiles get a DVE multiply
with host-computed exp(S*bias) tiles, every other tile folds its constant
bias factor into the v rows (applied during the v-projection PSUM->SBUF copy).
The cross-attn band position varies per core, so all 4 cross k-tiles use the
DVE multiply path.
"""

import sys

sys.path.insert(0, "/opt/trn_rl_repo")

import numpy as np
import ml_dtypes

BF16 = ml_dtypes.bfloat16

P = 128
F = 1024
FC = 8  # feature chunks of 128
H = 16
D = 64
NCTX = 2048
NQ = 512
MCTX = 512
NB = 32
MAXD = 128
EPS = 1e-5
S = np.float32(D) ** np.float32(-0.5)
ROT = 512  # own q rows sit at rotated positions [512, 1024)
BAND0 = 3  # self-attn band tiles 3..8
NBAND = 6
SLOT = 65  # v head slot: 64 v cols + ones col
NKT = NCTX // P  # 16 self k-tiles
NKTC = MCTX // P  # 4 cross k-tiles

_CACHE = {}


def _bucket_np(rel):
    """T5 relative position bucket, non-causal. Must match reference.py."""
    rel = np.asarray(rel, np.int64)
    nb = NB // 2
    ret = (rel >= 0).astype(np.int32) * nb
    n = np.abs(rel)
    max_exact = nb // 2
    val_large = max_exact + (
        np.log(np.maximum(n, 1).astype(np.float32) / np.float32(max_exact))
        / np.float32(np.log(MAXD / max_exact))
        * np.float32(nb - max_exact)
    ).astype(np.int32)
    val_large = np.minimum(val_large, nb - 1)
    return ret + np.where(n < max_exact, n, val_large)


def _build_module():
    import concourse.bass as bass
    import concourse.tile as tile
    import concourse.mybir as mybir
    from concourse import bacc
    from contextlib import ExitStack

    f32 = mybir.dt.float32
    bf16 = mybir.dt.bfloat16
    AF = mybir.ActivationFunctionType
    ALU = mybir.AluOpType

    nc = bacc.Bacc("TRN2", target_bir_lowering=False, debug=False, num_devices=8)

    # ---- DRAM I/O ----
    d_xctx = nc.dram_tensor("x_ctx", [NCTX, F], f32, kind="ExternalInput").ap()
    d_ctx = nc.dram_tensor("ctx", [MCTX, F], f32, kind="ExternalInput").ap()
    dw = {}
    for nm in ("w_qT", "w_kT", "w_v", "w_o", "cw_qT", "cw_kT", "cw_v", "cw_o"):
        dw[nm] = nc.dram_tensor(nm, [FC, P, F], bf16, kind="ExternalInput").ap()
    # norm vectors, feat-chunked per partition: [vec, 128, FC]
    # order: 0=sa_gc 1=sa_bc 2=sa_g 3=sa_b 4=ca_g 5=ca_b 6=ca_gc 7=ca_bc
    d_nrm = nc.dram_tensor("nrm", [P, 8, FC], f32, kind="ExternalInput").ap()
    d_bo = nc.dram_tensor("bo2", [2, P, F], f32, kind="ExternalInput").ap()
    d_ebs = nc.dram_tensor(
        "eb_self", [H, NBAND, P, NQ], bf16, kind="ExternalInput"
    ).ap()
    d_ebc = nc.dram_tensor(
        "eb_cross", [H, NKTC, P, NQ], bf16, kind="ExternalInput"
    ).ap()
    d_ones = nc.dram_tensor("ones_map", [P, NKT, H], bf16, kind="ExternalInput").ap()
    d_onesc = nc.dram_tensor("ones_c", [P, NKTC, H], bf16, kind="ExternalInput").ap()
    d_ident = nc.dram_tensor("ident", [P, P], bf16, kind="ExternalInput").ap()
    d_y = nc.dram_tensor("y", [NQ, F], f32, kind="ExternalOutput").ap()
    d_x2 = nc.dram_tensor("x2d", [NQ, F], f32).ap()  # internal scratch

    es = ExitStack()
    tc = es.enter_context(tile.TileContext(nc))

    pool = lambda name, bufs, **kw: es.enter_context(
        tc.tile_pool(name=name, bufs=bufs, **kw)
    )
    px = pool("px", 2)  # x input tiles f32 [128,1024]
    pst = pool("pst", 6)  # small stats tiles
    pcent = pool("pcent", 4)  # centered bf16 [128,1024]
    pwres = pool("pwres", 3)  # weight slab halves [P,4,F] bf16
    pkT = pool("pkT", 2)  # kT chunks [128, 2048] bf16
    pattn = pool("pattn", 3)  # attn group tiles [128, 1024] bf16
    peb = pool("peb", 1)  # eb tiles bf16 [128,nband*512]
    pres = pool("pres", 2)  # residual tiles f32 [128,512]
    pyt = pool("pyt", 2)  # output tiles f32 [128,512]
    pden = pool("pden", 4)  # denom tiles
    pone = pool("pone", 2)  # ones map tiles
    pmm = pool("pmm", 2, space="PSUM")  # [128,512] proj/transpose psum: 2 banks
    psim = pool("psim", 2, space="PSUM")  # [128,1024] sim psum: 4 banks
    pav = pool("pav", 2, space="PSUM")  # [66,512] av psum: 2 banks
    pbig = pool("pbig", 1)  # persistent tensors (one slot per tag)

    def big(name, shape, dtype, tag=None):
        return pbig.tile(shape, dtype, tag=tag or name, name=name)

    # persistent SBUF (self phase; cross phase reuses the same tags)
    xnTc = big("xnTc", [P, FC, NCTX], bf16)
    xnTq = big("xnTq", [P, FC, NQ], bf16)
    qT = big("qT", [P, FC, NQ], bf16)
    vsb = big("vsb", [P, NKT, H * SLOT], bf16)
    oT = big("oT", [P, FC, NQ], bf16)
    bo_s = big("bo_s", [P, F], f32)
    bo_c = big("bo_c", [P, F], f32)
    ident = big("ident", [P, P], bf16)
    nrmv = big("nrmv", [P, 8, FC], f32)

    eps_t = big("eps", [P, 1], f32)
    nc.vector.memset(eps_t[:, :], float(EPS))
    ones64 = big("ones64", [1, D], f32)
    nc.vector.memset(ones64[:, :], 1.0)
    nc.sync.dma_start(out=ident[:, :], in_=d_ident[:, :])
    nc.sync.dma_start(out=bo_s[:, :], in_=d_bo[0])
    nc.sync.dma_start(out=bo_c[:, :], in_=d_bo[1])
    nc.sync.dma_start(out=nrmv[:, :, :], in_=d_nrm[:, :, :])

    def g_ap(vi, fc):
        return nrmv[:, vi, fc : fc + 1]

    # ---------------- layer norm + transpose ----------------
    def ln_transpose(x_dram, r0, nrows, affines, group_order=None):
        """LN rows [r0, r0+nrows) of x_dram; write transposed+affined copies.

        affines: list of (dst, vi_g, vi_b, src_lo, src_hi) — dst[:, fc, j]
        gets position src_lo+j, for src positions within [src_lo, src_hi).
        Positions are local (0 = row r0).
        """
        nt = nrows // P
        groups = list(range(0, nt, 4))
        if group_order is not None:
            groups = [g * 4 for g in group_order]
        for g0 in groups:
            gn = min(4, nt - g0)
            cents = []
            for it in range(g0, g0 + gn):
                xt = px.tile([P, F], f32, tag="xt")
                nc.sync.dma_start(
                    out=xt[:, :], in_=x_dram[r0 + it * P : r0 + (it + 1) * P, :]
                )
                st6 = pst.tile([P, 2, 6], f32, tag="st6")
                nc.vector.bn_stats(st6[:, 0, :], xt[:, 0:512])
                nc.vector.bn_stats(st6[:, 1, :], xt[:, 512:1024])
                mv = pst.tile([P, 2], f32, tag="mv")
                nc.vector.bn_aggr(mv[:, :], st6[:, :, :])
                std = pst.tile([P, 1], f32, tag="std")
                nc.scalar.activation(std[:, :], mv[:, 1:2], AF.Sqrt, bias=eps_t[:, 0:1])
                rstd = pst.tile([P, 1], f32, tag="rstd")
                nc.vector.reciprocal(rstd[:, :], std[:, :])
                nmr = pst.tile([P, 1], f32, tag="nmr")
                nc.vector.scalar_tensor_tensor(
                    nmr[:, :], mv[:, 0:1], -1.0, rstd[:, :], ALU.mult, ALU.mult
                )
                cent = pcent.tile([P, F], bf16, tag="cent")
                nc.scalar.activation(
                    cent[:, :],
                    xt[:, :],
                    AF.Identity,
                    bias=nmr[:, 0:1],
                    scale=rstd[:, 0:1],
                )
                cents.append(cent)
            base = g0 * P
            for fc in range(FC):
                ps = pmm.tile([P, 4 * P], bf16, tag="mm")
                for j in range(gn):
                    nc.tensor.transpose(
                        ps[:, j * P : (j + 1) * P],
                        cents[j][:, fc * P : (fc + 1) * P],
                        ident[:, :],
                    )
                for dst, vig, vib, lo, hi in affines:
                    lo2, hi2 = max(lo, base), min(hi, base + gn * P)
                    if lo2 >= hi2:
                        continue
                    nc.vector.tensor_scalar(
                        dst[:, fc, lo2 - lo : hi2 - lo],
                        ps[:, lo2 - base : hi2 - base],
                        g_ap(vig, fc),
                        g_ap(vib, fc),
                        ALU.mult,
                        ALU.add,
                    )

    # ---------------- projection helpers ----------------
    def load_slabs(w_dram, pw, tag):
        halves = []
        for i in range(2):
            ws = pw.tile([P, 4, F], bf16, tag=tag)
            nc.sync.dma_start(
                out=ws[:, :, :],
                in_=w_dram[4 * i : 4 * i + 4].rearrange("c p f -> p c f"),
            )
            halves.append(ws)
        return [halves[k // 4][:, k % 4, :] for k in range(FC)]

    def proj_T(dst, w_dram, src, n_cols):
        """dst[:, m, :] = w^T @ src  (both operands feat-major chunked)."""
        slabs = load_slabs(w_dram, pwres, "wres")
        for m in range(FC):
            for n0 in range(0, n_cols, 512):
                nn = min(512, n_cols - n0)
                ps = pmm.tile([P, 512], f32, tag="mm")
                for k in range(FC):
                    nc.tensor.matmul(
                        ps[:, 0:nn],
                        slabs[k][:, m * P : (m + 1) * P],
                        src[:, k, n0 : n0 + nn],
                        start=(k == 0),
                        stop=(k == FC - 1),
                    )
                nc.vector.tensor_copy(dst[:, m, n0 : n0 + nn], ps[:, 0:nn])

    def v_proj(dst, w_dram, src, nkt, use_fac, ones_dram, m_order=None):
        """dst [P, nkt, H*SLOT]: natural-orientation v with slotted heads."""
        slabs = load_slabs(w_dram, pwres, "wres")
        ones_t = pone.tile([P, nkt, H], bf16, tag="ones")
        nc.sync.dma_start(out=ones_t[:, :, :], in_=ones_dram[:, 0:nkt, :])
        morder = list(m_order) + [m for m in range(nkt) if m not in m_order] if m_order else list(range(nkt))
        morder = morder[:nkt]
        for n in range(2):  # vcol halves: heads 0-7 / 8-15
            for m in morder:
                ps = pmm.tile([P, 512], f32, tag="mm")
                for k in range(FC):
                    nc.tensor.matmul(
                        ps[:, :],
                        src[:, k, m * P : (m + 1) * P],
                        slabs[k][:, n * 512 : (n + 1) * 512],
                        start=(k == 0),
                        stop=(k == FC - 1),
                    )
                out_ap = dst[:, m, n * 8 * SLOT : (n + 1) * 8 * SLOT].rearrange(
                    "p (h s) -> p h s", h=8
                )[:, :, 0:D]
                in_ap = ps[:, :].rearrange("p (h s) -> p h s", h=8)
                if use_fac:
                    vf = ones_t[:, m, n * 8 : (n + 1) * 8].to_broadcast((P, 8, D))
                    nc.vector.tensor_tensor(out_ap, in_ap, vf, ALU.mult)
                else:
                    nc.vector.tensor_copy(out_ap, in_ap)
        nc.vector.tensor_copy(
            dst[:, 0:nkt, :].rearrange("p t (h s) -> p t h s", h=H)[:, :, :, D],
            ones_t[:, :, :],
        )

    def kT_mchunk(w_dram, src, m, nctx):
        """kT chunk [128, nctx] for kcol tile m (heads 2m, 2m+1)."""
        slabs = load_slabs(w_dram, pwres, "wres")
        kt = pkT.tile([P, NCTX], bf16, tag="kT")
        for n0 in range(0, nctx, 512):
            ps = pmm.tile([P, 512], f32, tag="mm")
            for k in range(FC):
                nc.tensor.matmul(
                    ps[:, :],
                    slabs[k][:, m * P : (m + 1) * P],
                    src[:, k, n0 : n0 + 512],
                    start=(k == 0),
                    stop=(k == FC - 1),
                )
            nc.vector.tensor_copy(kt[:, n0 : n0 + 512], ps[:, :])
        return kt

    def attention(h, kt, qT_t, vsb_t, oT_t, nkt, eb_dram, band_lo, band_hi):
        """One head: sim -> exp -> (band mul) -> av -> normalized oT slice."""
        po = 64 * (h % 2)
        mc = h // 2
        nband = band_hi - band_lo
        ebt = peb.tile([P, nband, 512], bf16, tag="ebt")
        nc.sync.dma_start(
            out=ebt[:, :, :], in_=eb_dram[h].rearrange("b p q -> p b q")
        )
        gtiles = []
        for g0 in range(0, nkt, 2):
            ps = psim.tile([P, 1024], f32, tag="sim")
            for j in range(2):
                kc = g0 + j
                nc.tensor.matmul(
                    ps[:, j * 512 : (j + 1) * 512],
                    kt[po : po + D, kc * P : (kc + 1) * P],
                    qT_t[po : po + D, mc, :],
                    start=True,
                    stop=True,
                )
            at = pattn.tile([P, 1024], bf16, tag="attn")
            nc.scalar.activation(at[:, :], ps[:, :], AF.Exp)
            for j in range(2):
                kc = g0 + j
                if band_lo <= kc < band_hi:
                    nc.vector.tensor_tensor(
                        at[:, j * 512 : (j + 1) * 512],
                        at[:, j * 512 : (j + 1) * 512],
                        ebt[:, kc - band_lo, :],
                        ALU.mult,
                    )
            gtiles.append(at)
        pso = pav.tile([D + 1, 512], f32, tag="av")
        for kc in range(nkt):
            nc.tensor.matmul(
                pso[:, :],
                vsb_t[:, kc, h * SLOT : h * SLOT + D + 1],
                gtiles[kc // 2][:, (kc % 2) * 512 : (kc % 2 + 1) * 512],
                start=(kc == 0),
                stop=(kc == nkt - 1),
            )
        f32r = mybir.dt.float32r
        den = pden.tile([1, 512], f32r, tag="den")
        with nc.allow_low_precision(reason="softmax denom reciprocal in f32r"):
            nc.vector.reciprocal(den[:, :], pso[D : D + 1, :])
        psb = pmm.tile([P, 512], f32, tag="mm")
        nc.tensor.matmul(
            psb[0:D, :],
            ones64[:, :].bitcast(f32r),
            den[:, :],
            start=True,
            stop=True,
        )
        denb = pden.tile([D, 512], f32, tag="denb")
        nc.scalar.copy(denb[:, :], psb[0:D, :])
        nc.vector.tensor_tensor(
            oT_t[po : po + D, mc, :], pso[0:D, :], denb[:, :], ALU.mult
        )

    def out_proj(oT_t, w_dram, bo_t, dst_fn):
        slabs = load_slabs(w_dram, pwres, "wres")
        for qm in range(4):
            for n in range(2):
                ps = pmm.tile([P, 512], f32, tag="mm")
                for k in range(FC):
                    nc.tensor.matmul(
                        ps[:, :],
                        oT_t[:, k, qm * P : (qm + 1) * P],
                        slabs[k][:, n * 512 : (n + 1) * 512],
                        start=(k == 0),
                        stop=(k == FC - 1),
                    )
                tmp = pyt.tile([P, 512], f32, tag="yt")
                nc.vector.tensor_tensor(
                    tmp[:, :], ps[:, :], bo_t[:, n * 512 : (n + 1) * 512], ALU.add
                )
                dst_fn(qm, n, tmp)

    # ================= self attention =================
    ln_transpose(
        d_xctx, 0, NCTX, [(xnTc, 0, 1, 0, NCTX), (xnTq, 2, 3, ROT, ROT + NQ)]
    )
    proj_T(qT, dw["w_qT"], xnTq, NQ)
    v_proj(vsb, dw["w_v"], xnTc, NKT, True, d_ones)

    wk_slabs = load_slabs(dw["w_kT"], pwk, "wk")
    for m in range(FC):
        kt = kT_mchunk(wk_slabs, xnTc, m, NCTX)
        for h in (2 * m, 2 * m + 1):
            attention(h, kt, qT, vsb, oT, NKT, d_ebs, BAND0, BAND0 + NBAND)

    def self_dst(qm, n, tmp):
        rt = pres.tile([P, 512], f32, tag="res")
        nc.sync.dma_start(
            out=rt[:, :],
            in_=d_xctx[
                ROT + qm * P : ROT + (qm + 1) * P, n * 512 : (n + 1) * 512
            ],
        )
        x2t = pyt.tile([P, 512], f32, tag="yt")
        nc.vector.tensor_tensor(x2t[:, :], tmp[:, :], rt[:, :], ALU.add)
        nc.sync.dma_start(
            out=d_x2[qm * P : (qm + 1) * P, n * 512 : (n + 1) * 512], in_=x2t[:, :]
        )

    out_proj(oT, dw["w_o"], bo_s, self_dst)

    # ================= cross attention =================
    ctxnT = big("ctxnT", [P, FC, MCTX], bf16, tag="xnTc")
    x2nT = big("x2nT", [P, FC, NQ], bf16, tag="xnTq")
    qTc = big("qTc", [P, FC, NQ], bf16, tag="qT")
    vc = big("vc", [P, NKTC, H * SLOT], bf16, tag="vsb")
    oTc = big("oTc", [P, FC, NQ], bf16, tag="oT")

    ln_transpose(d_ctx, 0, MCTX, [(ctxnT, 6, 7, 0, MCTX)])
    ln_transpose(d_x2, 0, NQ, [(x2nT, 4, 5, 0, NQ)])
    proj_T(qTc, dw["cw_qT"], x2nT, NQ)
    v_proj(vc, dw["cw_v"], ctxnT, NKTC, False, d_onesc)

    wkc_slabs = load_slabs(dw["cw_kT"], pwk, "wk")
    for m in range(FC):
        kt = kT_mchunk(wkc_slabs, ctxnT, m, MCTX)
        for h in (2 * m, 2 * m + 1):
            attention(h, kt, qTc, vc, oTc, NKTC, d_ebc, 0, NKTC)

    def cross_dst(qm, n, tmp):
        rt = pres.tile([P, 512], f32, tag="res")
        nc.sync.dma_start(
            out=rt[:, :], in_=d_x2[qm * P : (qm + 1) * P, n * 512 : (n + 1) * 512]
        )
        yt = pyt.tile([P, 512], f32, tag="yt")
        nc.vector.tensor_tensor(yt[:, :], tmp[:, :], rt[:, :], ALU.add)
        nc.sync.dma_start(
            out=d_y[qm * P : (qm + 1) * P, n * 512 : (n + 1) * 512], in_=yt[:, :]
        )

    out_proj(oTc, dw["cw_o"], bo_c, cross_dst)

    es.close()
    nc.compile()
    return nc


# ---------------------------------------------------------------------------
# host side
# ---------------------------------------------------------------------------


def _prep_shared(inputs):
    sh = {}
    for pre in ("sa", "ca"):
        wq = inputs[f"{pre}_wq"].astype(np.float32) * S
        wkv = inputs[f"{pre}_wkv"].astype(np.float32)
        wo = inputs[f"{pre}_wo"].astype(np.float32)
        sh[f"{pre}_wqT"] = np.ascontiguousarray(wq.reshape(FC, P, F).astype(BF16))
        sh[f"{pre}_wkT"] = np.ascontiguousarray(
            wkv[:, :F].reshape(FC, P, F).astype(BF16)
        )
        sh[f"{pre}_wv"] = np.ascontiguousarray(wkv[:, F:].reshape(FC, P, F).astype(BF16))
        sh[f"{pre}_wo"] = np.ascontiguousarray(wo.reshape(FC, P, F).astype(BF16))
    nrm = np.stack(
        [
            inputs["sa_normc_g"],
            inputs["sa_normc_b"],
            inputs["sa_norm_g"],
            inputs["sa_norm_b"],
            inputs["ca_norm_g"],
            inputs["ca_norm_b"],
            inputs["ca_normc_g"],
            inputs["ca_normc_b"],
        ]
    ).astype(np.float32)
    # [8, F] -> [8, P, FC]: value for (partition p, chunk c) = vec[c*128+p]
    sh["nrm"] = np.ascontiguousarray(nrm.reshape(8, FC, P).transpose(2, 0, 1))
    sh["bo2"] = np.ascontiguousarray(
        np.stack(
            [
                np.broadcast_to(inputs["sa_bo"].astype(np.float32), (P, F)),
                np.broadcast_to(inputs["ca_bo"].astype(np.float32), (P, F)),
            ]
        )
    )
    sh["ident"] = np.eye(P, dtype=np.float32).astype(BF16)
    sh["ones_c"] = np.ones((P, NKTC, H), BF16)
    return sh


def _prep_core(inputs, sh, b, qi):
    q0 = qi * NQ
    x = np.asarray(inputs["x"][b], np.float32)
    ctx = np.asarray(inputs["context"][b], np.float32)
    rot = (np.arange(NCTX) + (q0 - ROT)) % NCTX  # position i <- orig row rot[i]
    x_rot = np.ascontiguousarray(x[rot])

    emb_s = np.asarray(inputs["sa_rel"], np.float32)  # [32, 16]
    qcols = q0 + np.arange(NQ)
    pos = np.arange(BAND0 * P, (BAND0 + NBAND) * P)
    rel = rot[pos][:, None] - qcols[None, :]
    eb = np.exp(S * emb_s[_bucket_np(rel)])  # [NBAND*P, NQ, H]
    eb_self = np.ascontiguousarray(
        eb.transpose(2, 0, 1).reshape(H, NBAND, P, NQ).astype(BF16)
    )
    vfac = np.ones((NKT, H), np.float32)
    for t in list(range(0, BAND0)) + list(range(BAND0 + NBAND, NKT)):
        pos_t = np.arange(t * P, (t + 1) * P)
        rel_t = rot[pos_t][:, None] - qcols[None, :]
        bk = _bucket_np(rel_t)
        assert bk.min() == bk.max(), (b, qi, t, bk.min(), bk.max())
        vfac[t] = np.exp(S * emb_s[bk[0, 0]])
    ones_map = np.broadcast_to(vfac[None, :, :], (P, NKT, H)).astype(BF16)

    emb_c = np.asarray(inputs["ca_rel"], np.float32)
    relc = np.arange(MCTX)[:, None] - qcols[None, :] + (NCTX - MCTX)
    ebc = np.exp(S * emb_c[_bucket_np(relc)])
    eb_cross = np.ascontiguousarray(
        ebc.transpose(2, 0, 1).reshape(H, NKTC, P, NQ).astype(BF16)
    )

    return {
        "x_ctx": x_rot,
        "ctx": np.ascontiguousarray(ctx),
        "w_qT": sh["sa_wqT"],
        "w_kT": sh["sa_wkT"],
        "w_v": sh["sa_wv"],
        "w_o": sh["sa_wo"],
        "cw_qT": sh["ca_wqT"],
        "cw_kT": sh["ca_wkT"],
        "cw_v": sh["ca_wv"],
        "cw_o": sh["ca_wo"],
        "nrm": sh["nrm"],
        "bo2": sh["bo2"],
        "eb_self": eb_self,
        "eb_cross": eb_cross,
        "ones_map": np.ascontiguousarray(ones_map),
        "ones_c": sh["ones_c"],
        "ident": sh["ident"],
    }


def get_module():
    if "nc" not in _CACHE:
        _CACHE["nc"] = _build_module()
    return _CACHE["nc"]


def prep_all_cores(inputs):
    sh = _prep_shared(inputs)
    return [_prep_core(inputs, sh, *divmod(core, 4)) for core in range(8)]


def kernel(**inputs):
    from concourse.bass_utils import run_bass_kernel_spmd

    nc = get_module()
    in_maps = prep_all_cores(inputs)
    res = run_bass_kernel_spmd(nc, in_maps, list(range(8)))
    out = np.empty((2, NCTX, F), np.float32)
    for core in range(8):
        b, qi = divmod(core, 4)
        out[b, qi * NQ : (qi + 1) * NQ] = res.results[core]["y"]
    return out


# revision 32
# speedup vs baseline: 180.4967x; 1.0295x over previous
"""Trainium2 Bass kernel for nn_AttentionBlock (self-attn + cross-attn block).

Sharding: 8 cores = (2 batches) x (4 query-chunks of 512). Zero communication:
each core recomputes the kv projection for its batch (on-chip collectives are
far slower than the duplicated matmul work). Inside a core everything runs in
"transposed" orientation:

  xnT = LN(x)^T                [feat, ctx]   (PE transpose of centered rows,
                                              per-feature affine applied after)
  qT  = (wq*S)^T @ xnT_q       [qcol, nq]    (softmax scale folded into wq)
  kT  = wk^T @ xnT_c           [kcol, nk]    (streamed per 128-col chunk)
  v   = xnT_c^T @ wv           [nk, vcol]    (natural orientation, 66-wide head
                                              slots with a ones column at 64)
  simT[k, q] = kT_h (stationary) x qT_h (moving)   -> PSUM f32
  attn = exp(simT)  (ACT, no max-subtraction: |S*sim| < ~4)
  attn *= exp(S*bias)   (DVE, only on the non-saturated diagonal band)
  avT[0:66, q] = sum_k v_slot[k, :] * attn[k, q]   (row 64 = softmax denom)
  oT[h*64+d, q] = avT[d, q] * recip(avT[64, q])
  out[q, f] = oT^T @ wo + bo + residual

T5 rel-pos bias enters multiplicatively: exp((sim+bias)*S) =
exp(S*sim)*exp(S*bias), and bias depends only on (k - q), saturating for
|k - q| >= 91. The host rotates the self-attn context rows per core so the
non-saturated band sits at fixed k-tiles 5..10; those tiles get a DVE multiply
with host-computed exp(S*bias) tiles, every other tile folds its constant
bias factor into the v rows (applied during the v-projection PSUM->SBUF copy).
The cross-attn band position varies per core, so all 4 cross k-tiles use the
DVE multiply path.
"""

import sys

sys.path.insert(0, "/opt/trn_rl_repo")

import numpy as np
import ml_dtypes

BF16 = ml_dtypes.bfloat16

P = 128
F = 1024
FC = 8  # feature chunks of 128
H = 16
D = 64
NCTX = 2048
NQ = 512
MCTX = 512
NB = 32
MAXD = 128
EPS = 1e-5
S = np.float32(D) ** np.float32(-0.5)
ROT = 512  # own q rows sit at rotated positions [512, 1024)
BAND0 = 3  # self-attn band tiles 3..8
NBAND = 6
SLOT = 65  # v head slot: 64 v cols + ones col
NKT = NCTX // P  # 16 self k-tiles
NKTC = MCTX // P  # 4 cross k-tiles

_CACHE = {}


def _bucket_np(rel):
    """T5 relative position bucket, non-causal. Must match reference.py."""
    rel = np.asarray(rel, np.int64)
    nb = NB // 2
    ret = (rel >= 0).astype(np.int32) * nb
    n = np.abs(rel)
    max_exact = nb // 2
    val_large = max_exact + (
        np.log(np.maximum(n, 1).astype(np.float32) / np.float32(max_exact))
        / np.float32(np.log(MAXD / max_exact))
        * np.float32(nb - max_exact)
    ).astype(np.int32)
    val_large = np.minimum(val_large, nb - 1)
    return ret + np.where(n < max_exact, n, val_large)


def _build_module():
    import concourse.bass as bass
    import concourse.tile as tile
    import concourse.mybir as mybir
    from concourse import bacc
    from contextlib import ExitStack

    f32 = mybir.dt.float32
    bf16 = mybir.dt.bfloat16
    AF = mybir.ActivationFunctionType
    ALU = mybir.AluOpType

    nc = bacc.Bacc("TRN2", target_bir_lowering=False, debug=False, num_devices=8)

    # ---- DRAM I/O ----
    d_xctx = nc.dram_tensor("x_ctx", [NCTX, F], f32, kind="ExternalInput").ap()
    d_ctx = nc.dram_tensor("ctx", [MCTX, F], f32, kind="ExternalInput").ap()
    dw = {}
    for nm in ("w_qT", "w_kT", "w_v", "w_o", "cw_qT", "cw_kT", "cw_v", "cw_o"):
        dw[nm] = nc.dram_tensor(nm, [FC, P, F], bf16, kind="ExternalInput").ap()
    # norm vectors, feat-chunked per partition: [vec, 128, FC]
    # order: 0=sa_gc 1=sa_bc 2=sa_g 3=sa_b 4=ca_g 5=ca_b 6=ca_gc 7=ca_bc
    d_nrm = nc.dram_tensor("nrm", [P, 8, FC], f32, kind="ExternalInput").ap()
    d_bo = nc.dram_tensor("bo2", [2, P, F], f32, kind="ExternalInput").ap()
    d_ebs = nc.dram_tensor(
        "eb_self", [H, NBAND, P, NQ], bf16, kind="ExternalInput"
    ).ap()
    d_ebc = nc.dram_tensor(
        "eb_cross", [H, NKTC, P, NQ], bf16, kind="ExternalInput"
    ).ap()
    d_ones = nc.dram_tensor("ones_map", [P, NKT, H], bf16, kind="ExternalInput").ap()
    d_onesc = nc.dram_tensor("ones_c", [P, NKTC, H], bf16, kind="ExternalInput").ap()
    d_ident = nc.dram_tensor("ident", [P, P], bf16, kind="ExternalInput").ap()
    d_y = nc.dram_tensor("y", [NQ, F], f32, kind="ExternalOutput").ap()
    d_x2 = nc.dram_tensor("x2d", [NQ, F], f32).ap()  # internal scratch

    es = ExitStack()
    tc = es.enter_context(tile.TileContext(nc))

    pool = lambda name, bufs, **kw: es.enter_context(
        tc.tile_pool(name=name, bufs=bufs, **kw)
    )
    px = pool("px", 3)  # x input tiles f32 [128,1024]
    pst = pool("pst", 6)  # small stats tiles
    pcent = pool("pcent", 5)  # centered bf16 [128,1024]
    pwres = pool("pwres", 3)  # weight slab halves [P,4,F] bf16
    pkT = pool("pkT", 2)  # kT chunks [128, 2048] bf16
    pattn = pool("pattn", 4)  # attn group tiles [128, 1024] bf16
    peb = pool("peb", 1)  # eb tiles bf16 [128,nband*512]
    pres = pool("pres", 2)  # residual tiles f32 [128,512]
    pyt = pool("pyt", 2)  # output tiles f32 [128,512]
    pden = pool("pden", 2)  # denom tiles
    pone = pool("pone", 2)  # ones map tiles
    pmm = pool("pmm", 2, space="PSUM")  # [128,512] proj/transpose psum: 2 banks
    psim = pool("psim", 2, space="PSUM")  # [128,1024] sim psum: 4 banks
    pav = pool("pav", 2, space="PSUM")  # [66,512] av psum: 2 banks
    pbig = pool("pbig", 1)  # persistent tensors (one slot per tag)

    def big(name, shape, dtype, tag=None):
        return pbig.tile(shape, dtype, tag=tag or name, name=name)

    # persistent SBUF (self phase; cross phase reuses the same tags)
    xnTc = big("xnTc", [P, FC, NCTX], bf16)
    xnTq = big("xnTq", [P, FC, NQ], bf16)
    qT = big("qT", [P, FC, NQ], bf16)
    vsb = big("vsb", [P, NKT, H * SLOT], bf16)
    oT = big("oT", [P, FC, NQ], bf16)
    bo_s = big("bo_s", [P, F], f32)
    bo_c = big("bo_c", [P, F], f32)
    ident = big("ident", [P, P], bf16)
    nrmv = big("nrmv", [P, 8, FC], f32)

    eps_t = big("eps", [P, 1], f32)
    nc.vector.memset(eps_t[:, :], float(EPS))
    ones64 = big("ones64", [1, D], f32)
    nc.vector.memset(ones64[:, :], 1.0)
    nc.sync.dma_start(out=ident[:, :], in_=d_ident[:, :])
    nc.sync.dma_start(out=bo_s[:, :], in_=d_bo[0])
    nc.sync.dma_start(out=bo_c[:, :], in_=d_bo[1])
    nc.sync.dma_start(out=nrmv[:, :, :], in_=d_nrm[:, :, :])

    def g_ap(vi, fc):
        return nrmv[:, vi, fc : fc + 1]

    # ---------------- layer norm + transpose ----------------
    def ln_transpose(x_dram, r0, nrows, affines, group_order=None):
        """LN rows [r0, r0+nrows) of x_dram; write transposed+affined copies.

        affines: list of (dst, vi_g, vi_b, src_lo, src_hi) — dst[:, fc, j]
        gets position src_lo+j, for src positions within [src_lo, src_hi).
        Positions are local (0 = row r0).
        """
        nt = nrows // P
        groups = list(range(0, nt, 4))
        if group_order is not None:
            groups = [g * 4 for g in group_order]
        for g0 in groups:
            gn = min(4, nt - g0)
            cents = []
            for it in range(g0, g0 + gn):
                xt = px.tile([P, F], f32, tag="xt")
                nc.sync.dma_start(
                    out=xt[:, :], in_=x_dram[r0 + it * P : r0 + (it + 1) * P, :]
                )
                st6 = pst.tile([P, 2, 6], f32, tag="st6")
                nc.vector.bn_stats(st6[:, 0, :], xt[:, 0:512])
                nc.vector.bn_stats(st6[:, 1, :], xt[:, 512:1024])
                mv = pst.tile([P, 2], f32, tag="mv")
                nc.vector.bn_aggr(mv[:, :], st6[:, :, :])
                std = pst.tile([P, 1], f32, tag="std")
                nc.scalar.activation(std[:, :], mv[:, 1:2], AF.Sqrt, bias=eps_t[:, 0:1])
                rstd = pst.tile([P, 1], f32, tag="rstd")
                nc.vector.reciprocal(rstd[:, :], std[:, :])
                nmr = pst.tile([P, 1], f32, tag="nmr")
                nc.vector.scalar_tensor_tensor(
                    nmr[:, :], mv[:, 0:1], -1.0, rstd[:, :], ALU.mult, ALU.mult
                )
                cent = pcent.tile([P, F], bf16, tag="cent")
                nc.scalar.activation(
                    cent[:, :],
                    xt[:, :],
                    AF.Identity,
                    bias=nmr[:, 0:1],
                    scale=rstd[:, 0:1],
                )
                cents.append(cent)
            base = g0 * P
            for fc in range(FC):
                ps = pmm.tile([P, 4 * P], bf16, tag="mm")
                for j in range(gn):
                    nc.tensor.transpose(
                        ps[:, j * P : (j + 1) * P],
                        cents[j][:, fc * P : (fc + 1) * P],
                        ident[:, :],
                    )
                for dst, vig, vib, lo, hi in affines:
                    lo2, hi2 = max(lo, base), min(hi, base + gn * P)
                    if lo2 >= hi2:
                        continue
                    nc.vector.tensor_scalar(
                        dst[:, fc, lo2 - lo : hi2 - lo],
                        ps[:, lo2 - base : hi2 - base],
                        g_ap(vig, fc),
                        g_ap(vib, fc),
                        ALU.mult,
                        ALU.add,
                    )

    # ---------------- projection helpers ----------------
    def load_slabs(w_dram, pw, tag):
        halves = []
        for i in range(2):
            ws = pw.tile([P, 4, F], bf16, tag=tag)
            nc.sync.dma_start(
                out=ws[:, :, :],
                in_=w_dram[4 * i : 4 * i + 4].rearrange("c p f -> p c f"),
            )
            halves.append(ws)
        return [halves[k // 4][:, k % 4, :] for k in range(FC)]

    def proj_T(dst, w_dram, src, n_cols):
        """dst[:, m, :] = w^T @ src  (both operands feat-major chunked)."""
        slabs = load_slabs(w_dram, pwres, "wres")
        for m in range(FC):
            for n0 in range(0, n_cols, 512):
                nn = min(512, n_cols - n0)
                ps = pmm.tile([P, 512], f32, tag="mm")
                for k in range(FC):
                    nc.tensor.matmul(
                        ps[:, 0:nn],
                        slabs[k][:, m * P : (m + 1) * P],
                        src[:, k, n0 : n0 + nn],
                        start=(k == 0),
                        stop=(k == FC - 1),
                    )
                nc.vector.tensor_copy(dst[:, m, n0 : n0 + nn], ps[:, 0:nn])

    def v_proj(dst, w_dram, src, nkt, use_fac, ones_dram, m_order=None):
        """dst [P, nkt, H*SLOT]: natural-orientation v with slotted heads."""
        slabs = load_slabs(w_dram, pwres, "wres")
        ones_t = pone.tile([P, nkt, H], bf16, tag="ones")
        nc.sync.dma_start(out=ones_t[:, :, :], in_=ones_dram[:, 0:nkt, :])
        morder = list(m_order) + [m for m in range(nkt) if m not in m_order] if m_order else list(range(nkt))
        morder = morder[:nkt]
        for n in range(2):  # vcol halves: heads 0-7 / 8-15
            for m in morder:
                ps = pmm.tile([P, 512], f32, tag="mm")
                for k in range(FC):
                    nc.tensor.matmul(
                        ps[:, :],
                        src[:, k, m * P : (m + 1) * P],
                        slabs[k][:, n * 512 : (n + 1) * 512],
                        start=(k == 0),
                        stop=(k == FC - 1),
                    )
                out_ap = dst[:, m, n * 8 * SLOT : (n + 1) * 8 * SLOT].rearrange(
                    "p (h s) -> p h s", h=8
                )[:, :, 0:D]
                in_ap = ps[:, :].rearrange("p (h s) -> p h s", h=8)
                if use_fac:
                    vf = ones_t[:, m, n * 8 : (n + 1) * 8].to_broadcast((P, 8, D))
                    nc.vector.tensor_tensor(out_ap, in_ap, vf, ALU.mult)
                else:
                    nc.vector.tensor_copy(out_ap, in_ap)
        nc.vector.tensor_copy(
            dst[:, 0:nkt, :].rearrange("p t (h s) -> p t h s", h=H)[:, :, :, D],
            ones_t[:, :, :],
        )

    def kT_mchunk(w_dram, src, m, nctx):
        """kT chunk [128, nctx] for kcol tile m (heads 2m, 2m+1)."""
        slabs = load_slabs(w_dram, pwres, "wres")
        kt = pkT.tile([P, NCTX], bf16, tag="kT")
        for n0 in range(0, nctx, 512):
            ps = pmm.tile([P, 512], f32, tag="mm")
            for k in range(FC):
                nc.tensor.matmul(
                    ps[:, :],
                    slabs[k][:, m * P : (m + 1) * P],
                    src[:, k, n0 : n0 + 512],
                    start=(k == 0),
                    stop=(k == FC - 1),
                )
            nc.vector.tensor_copy(kt[:, n0 : n0 + 512], ps[:, :])
        return kt

    def attention(h, kt, qT_t, vsb_t, oT_t, nkt, eb_dram, band_lo, band_hi):
        """One head: sim -> exp -> (band mul) -> av -> normalized oT slice."""
        po = 64 * (h % 2)
        mc = h // 2
        nband = band_hi - band_lo
        ebt = peb.tile([P, nband, 512], bf16, tag="ebt")
        nc.sync.dma_start(
            out=ebt[:, :, :], in_=eb_dram[h].rearrange("b p q -> p b q")
        )
        gtiles = []
        for g0 in range(0, nkt, 2):
            ps = psim.tile([P, 1024], f32, tag="sim")
            for j in range(2):
                kc = g0 + j
                nc.tensor.matmul(
                    ps[:, j * 512 : (j + 1) * 512],
                    kt[po : po + D, kc * P : (kc + 1) * P],
                    qT_t[po : po + D, mc, :],
                    start=True,
                    stop=True,
                )
            at = pattn.tile([P, 1024], bf16, tag="attn")
            nc.scalar.activation(at[:, :], ps[:, :], AF.Exp)
            for j in range(2):
                kc = g0 + j
                if band_lo <= kc < band_hi:
                    nc.vector.tensor_tensor(
                        at[:, j * 512 : (j + 1) * 512],
                        at[:, j * 512 : (j + 1) * 512],
                        ebt[:, kc - band_lo, :],
                        ALU.mult,
                    )
            gtiles.append(at)
        pso = pav.tile([D + 1, 512], f32, tag="av")
        for kc in range(nkt):
            nc.tensor.matmul(
                pso[:, :],
                vsb_t[:, kc, h * SLOT : h * SLOT + D + 1],
                gtiles[kc // 2][:, (kc % 2) * 512 : (kc % 2 + 1) * 512],
                start=(kc == 0),
                stop=(kc == nkt - 1),
            )
        f32r = mybir.dt.float32r
        den = pden.tile([1, 512], f32r, tag="den")
        with nc.allow_low_precision(reason="softmax denom reciprocal in f32r"):
            nc.vector.reciprocal(den[:, :], pso[D : D + 1, :])
        psb = pmm.tile([P, 512], f32, tag="mm")
        nc.tensor.matmul(
            psb[0:D, :],
            ones64[:, :].bitcast(f32r),
            den[:, :],
            start=True,
            stop=True,
        )
        denb = pden.tile([D, 512], f32, tag="denb")
        nc.vector.tensor_copy(denb[:, :], psb[0:D, :])
        nc.vector.tensor_tensor(
            oT_t[po : po + D, mc, :], pso[0:D, :], denb[:, :], ALU.mult
        )

    def out_proj(oT_t, w_dram, bo_t, dst_fn):
        slabs = load_slabs(w_dram, pwres, "wres")
        for qm in range(4):
            for n in range(2):
                ps = pmm.tile([P, 512], f32, tag="mm")
                for k in range(FC):
                    nc.tensor.matmul(
                        ps[:, :],
                        oT_t[:, k, qm * P : (qm + 1) * P],
                        slabs[k][:, n * 512 : (n + 1) * 512],
                        start=(k == 0),
                        stop=(k == FC - 1),
                    )
                tmp = pyt.tile([P, 512], f32, tag="yt")
                nc.vector.tensor_tensor(
                    tmp[:, :], ps[:, :], bo_t[:, n * 512 : (n + 1) * 512], ALU.add
                )
                dst_fn(qm, n, tmp)

    # ================= self attention =================
    ln_transpose(
        d_xctx, 0, NCTX, [(xnTc, 0, 1, 0, NCTX), (xnTq, 2, 3, ROT, ROT + NQ)]
    )
    proj_T(qT, dw["w_qT"], xnTq, NQ)
    v_proj(vsb, dw["w_v"], xnTc, NKT, True, d_ones)

    wk_slabs = load_slabs(dw["w_kT"], pwk, "wk")
    for m in range(FC):
        kt = kT_mchunk(wk_slabs, xnTc, m, NCTX)
        for h in (2 * m, 2 * m + 1):
            attention(h, kt, qT, vsb, oT, NKT, d_ebs, BAND0, BAND0 + NBAND)

    def self_dst(qm, n, tmp):
        rt = pres.tile([P, 512], f32, tag="res")
        nc.sync.dma_start(
            out=rt[:, :],
            in_=d_xctx[
                ROT + qm * P : ROT + (qm + 1) * P, n * 512 : (n + 1) * 512
            ],
        )
        x2t = pyt.tile([P, 512], f32, tag="yt")
        nc.vector.tensor_tensor(x2t[:, :], tmp[:, :], rt[:, :], ALU.add)
        nc.sync.dma_start(
            out=d_x2[qm * P : (qm + 1) * P, n * 512 : (n + 1) * 512], in_=x2t[:, :]
        )

    out_proj(oT, dw["w_o"], bo_s, self_dst)

    # ================= cross attention =================
    ctxnT = big("ctxnT", [P, FC, MCTX], bf16, tag="xnTc")
    x2nT = big("x2nT", [P, FC, NQ], bf16, tag="xnTq")
    qTc = big("qTc", [P, FC, NQ], bf16, tag="qT")
    vc = big("vc", [P, NKTC, H * SLOT], bf16, tag="vsb")
    oTc = big("oTc", [P, FC, NQ], bf16, tag="oT")

    ln_transpose(d_ctx, 0, MCTX, [(ctxnT, 6, 7, 0, MCTX)])
    ln_transpose(d_x2, 0, NQ, [(x2nT, 4, 5, 0, NQ)])
    proj_T(qTc, dw["cw_qT"], x2nT, NQ)
    v_proj(vc, dw["cw_v"], ctxnT, NKTC, False, d_onesc)

    wkc_slabs = load_slabs(dw["cw_kT"], pwk, "wk")
    for m in range(FC):
        kt = kT_mchunk(wkc_slabs, ctxnT, m, MCTX)
        for h in (2 * m, 2 * m + 1):
            attention(h, kt, qTc, vc, oTc, NKTC, d_ebc, 0, NKTC)

    def cross_dst(qm, n, tmp):
        rt = pres.tile([P, 512], f32, tag="res")
        nc.sync.dma_start(
            out=rt[:, :], in_=d_x2[qm * P : (qm + 1) * P, n * 512 : (n + 1) * 512]
        )
        yt = pyt.tile([P, 512], f32, tag="yt")
        nc.vector.tensor_tensor(yt[:, :], tmp[:, :], rt[:, :], ALU.add)
        nc.sync.dma_start(
            out=d_y[qm * P : (qm + 1) * P, n * 512 : (n + 1) * 512], in_=yt[:, :]
        )

    out_proj(oTc, dw["cw_o"], bo_c, cross_dst)

    es.close()
    nc.compile()
    return nc


# ---------------------------------------------------------------------------
# host side
# ---------------------------------------------------------------------------


def _prep_shared(inputs):
    sh = {}
    for pre in ("sa", "ca"):
        wq = inputs[f"{pre}_wq"].astype(np.float32) * S
        wkv = inputs[f"{pre}_wkv"].astype(np.float32)
        wo = inputs[f"{pre}_wo"].astype(np.float32)
        sh[f"{pre}_wqT"] = np.ascontiguousarray(wq.reshape(FC, P, F).astype(BF16))
        sh[f"{pre}_wkT"] = np.ascontiguousarray(
            wkv[:, :F].reshape(FC, P, F).astype(BF16)
        )
        sh[f"{pre}_wv"] = np.ascontiguousarray(wkv[:, F:].reshape(FC, P, F).astype(BF16))
        sh[f"{pre}_wo"] = np.ascontiguousarray(wo.reshape(FC, P, F).astype(BF16))
    nrm = np.stack(
        [
            inputs["sa_normc_g"],
            inputs["sa_normc_b"],
            inputs["sa_norm_g"],
            inputs["sa_norm_b"],
            inputs["ca_norm_g"],
            inputs["ca_norm_b"],
            inputs["ca_normc_g"],
            inputs["ca_normc_b"],
        ]
    ).astype(np.float32)
    # [8, F] -> [8, P, FC]: value for (partition p, chunk c) = vec[c*128+p]
    sh["nrm"] = np.ascontiguousarray(nrm.reshape(8, FC, P).transpose(2, 0, 1))
    sh["bo2"] = np.ascontiguousarray(
        np.stack(
            [
                np.broadcast_to(inputs["sa_bo"].astype(np.float32), (P, F)),
                np.broadcast_to(inputs["ca_bo"].astype(np.float32), (P, F)),
            ]
        )
    )
    sh["ident"] = np.eye(P, dtype=np.float32).astype(BF16)
    sh["ones_c"] = np.ones((P, NKTC, H), BF16)
    return sh


def _prep_core(inputs, sh, b, qi):
    q0 = qi * NQ
    x = np.asarray(inputs["x"][b], np.float32)
    ctx = np.asarray(inputs["context"][b], np.float32)
    rot = (np.arange(NCTX) + (q0 - ROT)) % NCTX  # position i <- orig row rot[i]
    x_rot = np.ascontiguousarray(x[rot])

    emb_s = np.asarray(inputs["sa_rel"], np.float32)  # [32, 16]
    qcols = q0 + np.arange(NQ)
    pos = np.arange(BAND0 * P, (BAND0 + NBAND) * P)
    rel = rot[pos][:, None] - qcols[None, :]
    eb = np.exp(S * emb_s[_bucket_np(rel)])  # [NBAND*P, NQ, H]
    eb_self = np.ascontiguousarray(
        eb.transpose(2, 0, 1).reshape(H, NBAND, P, NQ).astype(BF16)
    )
    vfac = np.ones((NKT, H), np.float32)
    for t in list(range(0, BAND0)) + list(range(BAND0 + NBAND, NKT)):
        pos_t = np.arange(t * P, (t + 1) * P)
        rel_t = rot[pos_t][:, None] - qcols[None, :]
        bk = _bucket_np(rel_t)
        assert bk.min() == bk.max(), (b, qi, t, bk.min(), bk.max())
        vfac[t] = np.exp(S * emb_s[bk[0, 0]])
    ones_map = np.broadcast_to(vfac[None, :, :], (P, NKT, H)).astype(BF16)

    emb_c = np.asarray(inputs["ca_rel"], np.float32)
    relc = np.arange(MCTX)[:, None] - qcols[None, :] + (NCTX - MCTX)
    ebc = np.exp(S * emb_c[_bucket_np(relc)])
    eb_cross = np.ascontiguousarray(
        ebc.transpose(2, 0, 1).reshape(H, NKTC, P, NQ).astype(BF16)
    )

    return {
        "x_ctx": x_rot,
        "ctx": np.ascontiguousarray(ctx),
        "w_qT": sh["sa_wqT"],
        "w_kT": sh["sa_wkT"],
        "w_v": sh["sa_wv"],
        "w_o": sh["sa_wo"],
        "cw_qT": sh["ca_wqT"],
        "cw_kT": sh["ca_wkT"],
        "cw_v": sh["ca_wv"],
        "cw_o": sh["ca_wo"],
        "nrm": sh["nrm"],
        "bo2": sh["bo2"],
        "eb_self": eb_self,
        "eb_cross": eb_cross,
        "ones_map": np.ascontiguousarray(ones_map),
        "ones_c": sh["ones_c"],
        "ident": sh["ident"],
    }


def get_module():
    if "nc" not in _CACHE:
        _CACHE["nc"] = _build_module()
    return _CACHE["nc"]


def prep_all_cores(inputs):
    sh = _prep_shared(inputs)
    return [_prep_core(inputs, sh, *divmod(core, 4)) for core in range(8)]


def kernel(**inputs):
    from concourse.bass_utils import run_bass_kernel_spmd

    nc = get_module()
    in_maps = prep_all_cores(inputs)
    res = run_bass_kernel_spmd(nc, in_maps, list(range(8)))
    out = np.empty((2, NCTX, F), np.float32)
    for core in range(8):
        b, qi = divmod(core, 4)
        out[b, qi * NQ : (qi + 1) * NQ] = res.results[core]["y"]
    return out
